# revision 5
# baseline (speedup 1.0000x reference)
"""Trainium2 Bass kernel for nn_CrossAttention (B=4, N=2048, E=768, H=8).

Sharding: 8 cores = 4 batches x 2 head-groups (4 heads of 96 dims each).
Each core computes its batch's attention for its 4 heads plus the partial
output projection; the host sums the two head-group partials per batch and
adds bo.

Per-core dataflow (all feature-major, no on-device transposes):
  K^T_h [96,2048] = Wk_h @ x_kv^T        (lhsT = Wk^T e-tiles, rhs = x_kv^T)
  Q^T_h [96,2048] = Wq_h @ x_q^T
  V     [128tok,4,97] tiles (col 96 = ones -> rowsums ride along matmul)
  S^T   [128kv,1024q] = K_h @ Q_h^T      (lhsT = K^T slice, rhs = Q^T)
  P^T   = exp(S^T/sqrt(768))             (ScalarE, PSUM->SBUF)
  O^T   [97,1024] += V_ext^T @ P^T       (lhsT = V tile, rhs = P^T)
  attn^T = O^T[0:96] * bcast(1/O^T[96])  (reciprocal + PE broadcast + DVE mul)
  out   [128q,768] += attn^T_h.T @ Wo^T_h  (partial; host adds group pairs + bo)
"""

import os
import sys
import types

import numpy as np

# ---------------------------------------------------------------------------
# NTFF profile hook (the agent image's antenv lacks axon_hooks; degrade OK)
# ---------------------------------------------------------------------------
def _install_ntff_hook():
    if "antenv.axon_hooks" in sys.modules:
        return
    try:
        hooks = types.ModuleType("antenv.axon_hooks")
        hooks._hook = None
        hooks.set_axon_ntff_profile_hook = lambda h: setattr(hooks, "_hook", h)
        hooks.get_axon_ntff_profile_hook = lambda: hooks._hook
        sys.modules["antenv.axon_hooks"] = hooks
        import antenv

        antenv.axon_hooks = hooks
        from trn_agent_boot.trn_boot import _ntff_profile_via_ctypes

        so = "/opt/axon/libaxon_pjrt.so"
        if os.path.exists(so):
            hooks.set_axon_ntff_profile_hook(_ntff_profile_via_ctypes(so))
    except Exception:
        pass


_install_ntff_hook()

import concourse.bacc as bacc
import concourse.tile as tile
import concourse.mybir as mybir
from concourse import bass_utils
from concourse.alu_op_type import AluOpType

F32 = mybir.dt.float32
F32R = mybir.dt.float32r
BF16 = mybir.dt.bfloat16

B = 4
NQ = 2048
NKV = 2048
E = 768
H_LOCAL = 4  # heads per core
HD = 96  # head dim
D = H_LOCAL * HD  # 384 local proj dim
ET = E // 128  # 6 contraction tiles
KV_T = NKV // 128  # 16 kv tiles
INV_SQRT_E = 1.0 / float(np.sqrt(np.float32(E)))

# matmul compute dtype: F32 (exact, 4 cyc/col) or F32R (1 cyc/col, reduced
# precision internally) -- storage is fp32 either way, flipped via bitcast.
MM_DT = F32 if os.environ.get("KERNEL_MM_DT", "f32") == "f32" else F32R


def _mm(ap):
    return ap.bitcast(MM_DT) if MM_DT is not F32 else ap


def build_nc():
    nc = bacc.Bacc("TRN2", target_bir_lowering=False, debug=False)

    xq_t = nc.dram_tensor("xq_t", [E, NQ], F32, kind="ExternalInput")
    xkv_t = nc.dram_tensor("xkv_t", [E, NKV], F32, kind="ExternalInput")
    wq_t = nc.dram_tensor("wq_t", [E, D], F32, kind="ExternalInput")
    wk_t = nc.dram_tensor("wk_t", [E, D], F32, kind="ExternalInput")
    wv_t = nc.dram_tensor("wv_t", [E, D], F32, kind="ExternalInput")
    wo_t = nc.dram_tensor("wo_t", [D, E], F32, kind="ExternalInput")
    bq = nc.dram_tensor("bq", [D], F32, kind="ExternalInput")
    bk = nc.dram_tensor("bk", [D], F32, kind="ExternalInput")
    bv = nc.dram_tensor("bv", [D], F32, kind="ExternalInput")
    out = nc.dram_tensor("out", [NQ, E], F32, kind="ExternalOutput")

    with tile.TileContext(nc) as tc:
        with (
            tc.tile_pool(name="persist", bufs=1) as persist,
            tc.tile_pool(name="psum_pf", bufs=2, space="PSUM") as ppf,
            tc.tile_pool(name="psum_s", bufs=2, space="PSUM") as pps,
            tc.tile_pool(name="psum_o", bufs=1, space="PSUM") as ppo,
        ):
            # persistent SBUF tensors
            KT = persist.tile([HD, H_LOCAL, NKV], F32)  # K^T per head
            QT = persist.tile([HD, H_LOCAL, NQ], F32)  # Q^T per head
            V = persist.tile([128, KV_T, H_LOCAL, HD + 1], F32)  # +ones col
            wo_sb = persist.tile([HD, H_LOCAL, E], F32)
            bq_sb = persist.tile([HD, H_LOCAL], F32)
            bk_sb = persist.tile([HD, H_LOCAL], F32)
            bv_sb = persist.tile([128, D], F32)
            ones_sb = persist.tile([1, HD], F32)

            nc.sync.dma_start(wo_sb[:], wo_t[:].rearrange("(h p) n -> p h n", p=HD))
            nc.sync.dma_start(bq_sb[:], bq[:].rearrange("(h p) -> p h", p=HD))
            nc.sync.dma_start(bk_sb[:], bk[:].rearrange("(h p) -> p h", p=HD))
            nc.sync.dma_start(bv_sb[:], bv[:].partition_broadcast(128))
            nc.vector.memset(ones_sb[:], 1.0)
            nc.vector.memset(V[:, :, :, HD : HD + 1], 1.0)

            attn = persist.tile([HD, H_LOCAL, NQ], F32)  # normalized attn^T

            # ---------------- projections ----------------
            with (
                tc.tile_pool(name="wpool", bufs=1) as wp,
                tc.tile_pool(name="xpool", bufs=6) as xp,
            ):
                wq_sb = wp.tile([128, ET, D], F32, tag="w")
                wk_sb = wp.tile([128, ET, D], F32, tag="wk")
                wv_sb = wp.tile([128, ET, D], F32, tag="wv")
                nc.sync.dma_start(wq_sb[:], wq_t[:].rearrange("(t p) n -> p t n", p=128))
                nc.sync.dma_start(wk_sb[:], wk_t[:].rearrange("(t p) n -> p t n", p=128))
                nc.sync.dma_start(wv_sb[:], wv_t[:].rearrange("(t p) n -> p t n", p=128))

                def load_x(dram, half):
                    tiles = []
                    for e in range(ET):
                        xt = xp.tile([128, 1024], F32, tag="x")
                        nc.sync.dma_start(
                            xt[:],
                            dram[e * 128 : (e + 1) * 128, half * 1024 : (half + 1) * 1024],
                        )
                        tiles.append(xt)
                    return tiles

                def proj_heads(x_tiles, w_sb, dst, b_sb, half):
                    # dst[:, h, half*1024 + n*512 ...] = w_h @ x^T + b
                    for h in range(H_LOCAL):
                        for n in range(2):
                            ps = ppf.tile([128, 512], F32, tag="pf")
                            for e in range(ET):
                                nc.tensor.matmul(
                                    ps[0:HD, :],
                                    _mm(w_sb[:, e, h * HD : (h + 1) * HD]),
                                    _mm(x_tiles[e][:, n * 512 : (n + 1) * 512]),
                                    start=(e == 0),
                                    stop=(e == ET - 1),
                                )
                            col = half * 1024 + n * 512
                            nc.vector.tensor_scalar_add(
                                out=dst[:, h, col : col + 512],
                                in0=ps[0:HD, :],
                                scalar1=b_sb[:, h : h + 1],
                            )

                for half in range(2):
                    xk_tiles = load_x(xkv_t, half)
                    proj_heads(xk_tiles, wk_sb, KT, bk_sb, half)
                    # V tiles for this half's tokens
                    for t in range(8):
                        tg = half * 8 + t
                        ps = ppf.tile([128, 512], F32, tag="pf")
                        for e in range(ET):
                            nc.tensor.matmul(
                                ps[:, 0:D],
                                _mm(xk_tiles[e][:, t * 128 : (t + 1) * 128]),
                                _mm(wv_sb[:, e, :]),
                                start=(e == 0),
                                stop=(e == ET - 1),
                            )
                        nc.vector.tensor_tensor(
                            out=V[:, tg, :, 0:HD],
                            in0=ps[:, 0:D].rearrange("p (h d) -> p h d", h=H_LOCAL),
                            in1=bv_sb.rearrange("p (h d) -> p h d", h=H_LOCAL),
                            op=AluOpType.add,
                        )
                for half in range(2):
                    xq_tiles = load_x(xq_t, half)
                    proj_heads(xq_tiles, wq_sb, QT, bq_sb, half)

            # ---------------- attention + output projection ----------------
            with tc.tile_pool(name="work", bufs=1) as work:
                out_pool = tc.tile_pool(name="outp", bufs=2)
                with out_pool as op_:
                    for qc in range(2):  # q chunks of 1024
                        for h in range(H_LOCAL):
                            po = ppo.tile([HD + 1, 1024], F32, tag="o")
                            for kv in range(KV_T):
                                s = pps.tile([128, 1024], F32, tag="s")
                                for n in range(2):
                                    nc.tensor.matmul(
                                        s[:, n * 512 : (n + 1) * 512],
                                        _mm(KT[:, h, kv * 128 : (kv + 1) * 128]),
                                        _mm(QT[:, h, qc * 1024 + n * 512 : qc * 1024 + (n + 1) * 512]),
                                        start=True,
                                        stop=True,
                                    )
                                p = work.tile([128, 1024], F32, tag="p", bufs=3)
                                nc.scalar.activation(
                                    p[:], s[:], mybir.ActivationFunctionType.Exp,
                                    scale=INV_SQRT_E,
                                )
                                for n in range(2):
                                    nc.tensor.matmul(
                                        po[:, n * 512 : (n + 1) * 512],
                                        _mm(V[:, kv, h, :]),
                                        _mm(p[:, n * 512 : (n + 1) * 512]),
                                        start=(kv == 0),
                                        stop=(kv == KV_T - 1),
                                    )
                            rc = work.tile([1, 1024], F32, tag="rc", bufs=2)
                            nc.vector.reciprocal(rc[:], po[HD : HD + 1, :])
                            o_sb = work.tile([HD, 1024], F32, tag="osb", bufs=2)
                            nc.scalar.copy(o_sb[:], po[0:HD, :])
                            for n in range(2):
                                bc = ppf.tile([128, 512], F32, tag="pf")
                                nc.tensor.matmul(
                                    bc[0:HD, :],
                                    _mm(ones_sb[:]),
                                    _mm(rc[:, n * 512 : (n + 1) * 512]),
                                    start=True,
                                    stop=True,
                                )
                                col = qc * 1024 + n * 512
                                nc.vector.tensor_mul(
                                    attn[:, h, col : col + 512],
                                    o_sb[:, n * 512 : (n + 1) * 512],
                                    bc[0:HD, :],
                                )
                        # output projection for this q chunk (8 tiles of 128)
                        for t in range(8):
                            qt = qc * 8 + t
                            fa = ppf.tile([128, 512], F32, tag="pf")
                            fb = ppf.tile([128, 512], F32, tag="pf")
                            for h in range(H_LOCAL):
                                nc.tensor.matmul(
                                    fa[:],
                                    _mm(attn[:, h, qt * 128 : (qt + 1) * 128]),
                                    _mm(wo_sb[:, h, 0:512]),
                                    start=(h == 0),
                                    stop=(h == H_LOCAL - 1),
                                )
                                nc.tensor.matmul(
                                    fb[:, 0:256],
                                    _mm(attn[:, h, qt * 128 : (qt + 1) * 128]),
                                    _mm(wo_sb[:, h, 512:768]),
                                    start=(h == 0),
                                    stop=(h == H_LOCAL - 1),
                                )
                            ob = op_.tile([128, E], F32, tag="ob")
                            nc.vector.tensor_copy(ob[:, 0:512], fa[:])
                            nc.vector.tensor_copy(ob[:, 512:768], fb[:, 0:256])
                            nc.sync.dma_start(
                                out[qt * 128 : (qt + 1) * 128, :], ob[:]
                            )

    nc.compile()
    return nc


_NC_CACHE = None


def kernel(x_query, x_kv, Wq, bq, Wk, bk, Wv, bv, Wo, bo):
    global _NC_CACHE
    x_query = np.asarray(x_query, dtype=np.float32)
    x_kv = np.asarray(x_kv, dtype=np.float32)
    Wq = np.asarray(Wq, dtype=np.float32)
    Wk = np.asarray(Wk, dtype=np.float32)
    Wv = np.asarray(Wv, dtype=np.float32)
    Wo = np.asarray(Wo, dtype=np.float32)
    bq = np.asarray(bq, dtype=np.float32)
    bk = np.asarray(bk, dtype=np.float32)
    bv = np.asarray(bv, dtype=np.float32)
    bo = np.asarray(bo, dtype=np.float32)

    if _NC_CACHE is None:
        _NC_CACHE = build_nc()
    nc = _NC_CACHE

    in_maps = []
    for c in range(8):
        b, g = divmod(c, 2)
        sl = slice(g * D, (g + 1) * D)
        in_maps.append(
            {
                "xq_t": np.ascontiguousarray(x_query[b].T),
                "xkv_t": np.ascontiguousarray(x_kv[b].T),
                "wq_t": np.ascontiguousarray(Wq[sl, :].T),
                "wk_t": np.ascontiguousarray(Wk[sl, :].T),
                "wv_t": np.ascontiguousarray(Wv[sl, :].T),
                "wo_t": np.ascontiguousarray(Wo[:, sl].T),
                "bq": np.ascontiguousarray(bq[sl]),
                "bk": np.ascontiguousarray(bk[sl]),
                "bv": np.ascontiguousarray(bv[sl]),
            }
        )

    trace = bool(int(os.environ.get("KERNEL_TRACE", "0")))
    res = bass_utils.run_bass_kernel_spmd(
        nc, in_maps, core_ids=list(range(8)), trace=trace
    )
    if trace:
        kernel.last_exec_time_ns = res.exec_time_ns
        kernel.last_results = res

    out = np.empty((B, NQ, E), dtype=np.float32)
    for b in range(B):
        out[b] = res.results[2 * b]["out"] + res.results[2 * b + 1]["out"] + bo
    return out


# revision 9
# speedup vs baseline: 2.5807x; 2.5807x over previous
"""Trainium2 Bass kernel for nn_CrossAttention (B=4, N=2048, E=768, H=8).

Sharding: 8 cores = 4 batches x 2 head-groups (4 heads of 96 dims each).
Each core computes its batch's attention for its 4 heads plus the partial
output projection; the host sums the two head-group partials per batch and
adds bo.

Per-core dataflow (all feature-major, no on-device transposes):
  K^T_h [96,2048] = Wk_h @ x_kv^T        (lhsT = Wk^T e-tiles, rhs = x_kv^T)
  Q^T_h [96,2048] = Wq_h @ x_q^T
  V     [128tok,4,97] tiles (col 96 = ones -> rowsums ride along matmul)
  S^T   [128kv,1024q] = K_h @ Q_h^T      (lhsT = K^T slice, rhs = Q^T)
  P^T   = exp(S^T/sqrt(768))             (ScalarE, PSUM->SBUF)
  O^T   [97,1024] += V_ext^T @ P^T       (lhsT = V tile, rhs = P^T)
  attn^T = O^T[0:96] * bcast(1/O^T[96])  (reciprocal + PE broadcast + DVE mul)
  out   [128q,768] += attn^T_h.T @ Wo^T_h  (partial; host adds group pairs + bo)
"""

import os
import sys
import types

import numpy as np

# ---------------------------------------------------------------------------
# NTFF profile hook (the agent image's antenv lacks axon_hooks; degrade OK)
# ---------------------------------------------------------------------------
def _install_ntff_hook():
    if "antenv.axon_hooks" in sys.modules:
        return
    try:
        hooks = types.ModuleType("antenv.axon_hooks")
        hooks._hook = None
        hooks.set_axon_ntff_profile_hook = lambda h: setattr(hooks, "_hook", h)
        hooks.get_axon_ntff_profile_hook = lambda: hooks._hook
        sys.modules["antenv.axon_hooks"] = hooks
        import antenv

        antenv.axon_hooks = hooks
        from trn_agent_boot.trn_boot import _ntff_profile_via_ctypes

        so = "/opt/axon/libaxon_pjrt.so"
        if os.path.exists(so):
            hooks.set_axon_ntff_profile_hook(_ntff_profile_via_ctypes(so))
    except Exception:
        pass


_install_ntff_hook()

import concourse.bacc as bacc
import concourse.tile as tile
import concourse.mybir as mybir
from concourse import bass_utils
from concourse.alu_op_type import AluOpType

F32 = mybir.dt.float32
F32R = mybir.dt.float32r
BF16 = mybir.dt.bfloat16

B = 4
NQ = 2048
NKV = 2048
E = 768
H_LOCAL = 4  # heads per core
HD = 96  # head dim
D = H_LOCAL * HD  # 384 local proj dim
ET = E // 128  # 6 contraction tiles
KV_T = NKV // 128  # 16 kv tiles
INV_SQRT_E = 1.0 / float(np.sqrt(np.float32(E)))

# matmul compute dtype: F32 (exact, 4 cyc/col) or F32R (1 cyc/col, reduced
# precision internally) -- storage is fp32 either way, flipped via bitcast.
MM_DT = F32 if os.environ.get("KERNEL_MM_DT", "f32") == "f32" else F32R


CDT = MM_DT  # compute dtype for matmul-input tiles


def _mm(ap):
    return ap


def _src(ap):
    # DRAM source APs bitcast to the compute dtype for direct DMA loads
    return ap.bitcast(CDT) if CDT is not F32 else ap


def build_nc():
    nc = bacc.Bacc("TRN2", target_bir_lowering=False, debug=False)

    xq_t = nc.dram_tensor("xq_t", [E, NQ], F32, kind="ExternalInput")
    xkv_t = nc.dram_tensor("xkv_t", [E, NKV], F32, kind="ExternalInput")
    wq_t = nc.dram_tensor("wq_t", [E, D], F32, kind="ExternalInput")
    wk_t = nc.dram_tensor("wk_t", [E, D], F32, kind="ExternalInput")
    wv_t = nc.dram_tensor("wv_t", [E, D], F32, kind="ExternalInput")
    wo_t = nc.dram_tensor("wo_t", [D, E], F32, kind="ExternalInput")
    bq = nc.dram_tensor("bq", [D], F32, kind="ExternalInput")
    bk = nc.dram_tensor("bk", [D], F32, kind="ExternalInput")
    bv = nc.dram_tensor("bv", [D], F32, kind="ExternalInput")
    out = nc.dram_tensor("out", [NQ, E], F32, kind="ExternalOutput")

    with tile.TileContext(nc) as tc:
        with (
            nc.allow_low_precision(reason="f32r matmul operand rounding"),
            tc.tile_pool(name="persist", bufs=1) as persist,
            tc.tile_pool(name="psum_pf", bufs=2, space="PSUM") as ppf,
            tc.tile_pool(name="psum_s", bufs=2, space="PSUM") as pps,
            tc.tile_pool(name="psum_o", bufs=1, space="PSUM") as ppo,
        ):
            # persistent SBUF tensors
            KT = persist.tile([HD, H_LOCAL, NKV], CDT)  # K^T per head
            QT = persist.tile([HD, H_LOCAL, NQ], CDT)  # Q^T per head
            V = persist.tile([128, KV_T, H_LOCAL, HD + 1], CDT)  # +ones col
            wo_sb = persist.tile([HD, H_LOCAL, E], CDT)
            bq_sb = persist.tile([HD, H_LOCAL], F32)
            bk_sb = persist.tile([HD, H_LOCAL], F32)
            bv_sb = persist.tile([128, D], F32)
            ones_sb = persist.tile([1, HD], CDT)

            nc.sync.dma_start(wo_sb[:], _src(wo_t[:].rearrange("(h p) n -> p h n", p=HD)))
            nc.sync.dma_start(bq_sb[:], bq[:].rearrange("(h p) -> p h", p=HD))
            nc.sync.dma_start(bk_sb[:], bk[:].rearrange("(h p) -> p h", p=HD))
            nc.sync.dma_start(bv_sb[:], bv[:].partition_broadcast(128))
            ones_f32 = persist.tile([128, HD], F32)
            nc.vector.memset(ones_f32[:], 1.0)
            nc.vector.tensor_copy(ones_sb[:], ones_f32[0:1, :])
            nc.vector.tensor_copy(
                V[:, :, :, HD : HD + 1],
                ones_f32[:, 0 : KV_T * H_LOCAL].rearrange(
                    "p (t h one) -> p t h one", t=KV_T, h=H_LOCAL, one=1
                ),
            )

            attn = persist.tile([HD, H_LOCAL, NQ], CDT)  # normalized attn^T

            # ---------------- projections ----------------
            with (
                tc.tile_pool(name="wpool", bufs=1) as wp,
                tc.tile_pool(name="xpool", bufs=6) as xp,
            ):
                wq_sb = wp.tile([128, ET, D], CDT, tag="w")
                wk_sb = wp.tile([128, ET, D], CDT, tag="wk")
                wv_sb = wp.tile([128, ET, D], CDT, tag="wv")
                nc.sync.dma_start(wq_sb[:], _src(wq_t[:].rearrange("(t p) n -> p t n", p=128)))
                nc.sync.dma_start(wk_sb[:], _src(wk_t[:].rearrange("(t p) n -> p t n", p=128)))
                nc.sync.dma_start(wv_sb[:], _src(wv_t[:].rearrange("(t p) n -> p t n", p=128)))

                def load_x(dram, half):
                    tiles = []
                    for e in range(ET):
                        xt = xp.tile([128, 1024], CDT, tag="x")
                        nc.sync.dma_start(
                            xt[:],
                            _src(dram[e * 128 : (e + 1) * 128, half * 1024 : (half + 1) * 1024]),
                        )
                        tiles.append(xt)
                    return tiles

                def proj_heads(x_tiles, w_sb, dst, b_sb, half):
                    # dst[:, h, half*1024 + n*512 ...] = w_h @ x^T + b
                    for h in range(H_LOCAL):
                        for n in range(2):
                            ps = ppf.tile([128, 512], F32, tag="pf")
                            for e in range(ET):
                                nc.tensor.matmul(
                                    ps[0:HD, :],
                                    _mm(w_sb[:, e, h * HD : (h + 1) * HD]),
                                    _mm(x_tiles[e][:, n * 512 : (n + 1) * 512]),
                                    start=(e == 0),
                                    stop=(e == ET - 1),
                                )
                            col = half * 1024 + n * 512
                            nc.vector.tensor_scalar_add(
                                out=dst[:, h, col : col + 512],
                                in0=ps[0:HD, :],
                                scalar1=b_sb[:, h : h + 1],
                            )

                for half in range(2):
                    xk_tiles = load_x(xkv_t, half)
                    proj_heads(xk_tiles, wk_sb, KT, bk_sb, half)
                    # V tiles for this half's tokens
                    for t in range(8):
                        tg = half * 8 + t
                        ps = ppf.tile([128, 512], F32, tag="pf")
                        for e in range(ET):
                            nc.tensor.matmul(
                                ps[:, 0:D],
                                _mm(xk_tiles[e][:, t * 128 : (t + 1) * 128]),
                                _mm(wv_sb[:, e, :]),
                                start=(e == 0),
                                stop=(e == ET - 1),
                            )
                        nc.vector.tensor_tensor(
                            out=V[:, tg, :, 0:HD],
                            in0=ps[:, 0:D].rearrange("p (h d) -> p h d", h=H_LOCAL),
                            in1=bv_sb.rearrange("p (h d) -> p h d", h=H_LOCAL),
                            op=AluOpType.add,
                        )
                for half in range(2):
                    xq_tiles = load_x(xq_t, half)
                    proj_heads(xq_tiles, wq_sb, QT, bq_sb, half)

            # ---------------- attention + output projection ----------------
            with tc.tile_pool(name="work", bufs=1) as work:
                out_pool = tc.tile_pool(name="outp", bufs=2)
                with out_pool as op_:
                    for qc in range(2):  # q chunks of 1024
                        for h in range(H_LOCAL):
                            po = ppo.tile([HD + 1, 1024], F32, tag="o")
                            for kv in range(KV_T):
                                s = pps.tile([128, 1024], F32, tag="s")
                                for n in range(2):
                                    nc.tensor.matmul(
                                        s[:, n * 512 : (n + 1) * 512],
                                        _mm(KT[:, h, kv * 128 : (kv + 1) * 128]),
                                        _mm(QT[:, h, qc * 1024 + n * 512 : qc * 1024 + (n + 1) * 512]),
                                        start=True,
                                        stop=True,
                                    )
                                p = work.tile([128, 1024], CDT, tag="p", bufs=3)
                                nc.scalar.activation(
                                    p[:], s[:], mybir.ActivationFunctionType.Exp,
                                    scale=INV_SQRT_E,
                                )
                                for n in range(2):
                                    nc.tensor.matmul(
                                        po[:, n * 512 : (n + 1) * 512],
                                        _mm(V[:, kv, h, :]),
                                        _mm(p[:, n * 512 : (n + 1) * 512]),
                                        start=(kv == 0),
                                        stop=(kv == KV_T - 1),
                                    )
                            rc = work.tile([1, 1024], CDT, tag="rc", bufs=2)
                            nc.vector.reciprocal(rc[:], po[HD : HD + 1, :])
                            o_sb = work.tile([HD, 1024], F32, tag="osb", bufs=2)
                            nc.scalar.copy(o_sb[:], po[0:HD, :])
                            for n in range(2):
                                bc = ppf.tile([128, 512], F32, tag="pf")
                                nc.tensor.matmul(
                                    bc[0:HD, :],
                                    _mm(ones_sb[:]),
                                    _mm(rc[:, n * 512 : (n + 1) * 512]),
                                    start=True,
                                    stop=True,
                                )
                                col = qc * 1024 + n * 512
                                nc.vector.tensor_mul(
                                    attn[:, h, col : col + 512],
                                    o_sb[:, n * 512 : (n + 1) * 512],
                                    bc[0:HD, :],
                                )
                        # output projection for this q chunk (8 tiles of 128)
                        for t in range(8):
                            qt = qc * 8 + t
                            fa = ppf.tile([128, 512], F32, tag="pf")
                            fb = ppf.tile([128, 512], F32, tag="pf")
                            for h in range(H_LOCAL):
                                nc.tensor.matmul(
                                    fa[:],
                                    _mm(attn[:, h, qt * 128 : (qt + 1) * 128]),
                                    _mm(wo_sb[:, h, 0:512]),
                                    start=(h == 0),
                                    stop=(h == H_LOCAL - 1),
                                )
                                nc.tensor.matmul(
                                    fb[:, 0:256],
                                    _mm(attn[:, h, qt * 128 : (qt + 1) * 128]),
                                    _mm(wo_sb[:, h, 512:768]),
                                    start=(h == 0),
                                    stop=(h == H_LOCAL - 1),
                                )
                            ob = op_.tile([128, E], F32, tag="ob")
                            nc.vector.tensor_copy(ob[:, 0:512], fa[:])
                            nc.vector.tensor_copy(ob[:, 512:768], fb[:, 0:256])
                            nc.sync.dma_start(
                                out[qt * 128 : (qt + 1) * 128, :], ob[:]
                            )

    nc.compile()
    return nc


_NC_CACHE = None


def kernel(x_query, x_kv, Wq, bq, Wk, bk, Wv, bv, Wo, bo):
    global _NC_CACHE
    x_query = np.asarray(x_query, dtype=np.float32)
    x_kv = np.asarray(x_kv, dtype=np.float32)
    Wq = np.asarray(Wq, dtype=np.float32)
    Wk = np.asarray(Wk, dtype=np.float32)
    Wv = np.asarray(Wv, dtype=np.float32)
    Wo = np.asarray(Wo, dtype=np.float32)
    bq = np.asarray(bq, dtype=np.float32)
    bk = np.asarray(bk, dtype=np.float32)
    bv = np.asarray(bv, dtype=np.float32)
    bo = np.asarray(bo, dtype=np.float32)

    if _NC_CACHE is None:
        _NC_CACHE = build_nc()
    nc = _NC_CACHE

    in_maps = []
    for c in range(8):
        b, g = divmod(c, 2)
        sl = slice(g * D, (g + 1) * D)
        in_maps.append(
            {
                "xq_t": np.ascontiguousarray(x_query[b].T),
                "xkv_t": np.ascontiguousarray(x_kv[b].T),
                "wq_t": np.ascontiguousarray(Wq[sl, :].T),
                "wk_t": np.ascontiguousarray(Wk[sl, :].T),
                "wv_t": np.ascontiguousarray(Wv[sl, :].T),
                "wo_t": np.ascontiguousarray(Wo[:, sl].T),
                "bq": np.ascontiguousarray(bq[sl]),
                "bk": np.ascontiguousarray(bk[sl]),
                "bv": np.ascontiguousarray(bv[sl]),
            }
        )

    trace = bool(int(os.environ.get("KERNEL_TRACE", "0")))
    res = bass_utils.run_bass_kernel_spmd(
        nc, in_maps, core_ids=list(range(8)), trace=trace
    )
    if trace:
        kernel.last_exec_time_ns = res.exec_time_ns
        kernel.last_results = res

    out = np.empty((B, NQ, E), dtype=np.float32)
    for b in range(B):
        out[b] = res.results[2 * b]["out"] + res.results[2 * b + 1]["out"] + bo
    return out
